# revision 1
# baseline (speedup 1.0000x reference)
"""Trainium2 Bass kernel: multi-head attention (b=4, s=2048, d_model=1024, h=16).

Sharding over 8 NeuronCores: 2-D (batch x head-half).
  core c -> batch c//2, head group c%2 (8 of 16 heads, qkv dims 512*g..512*g+512).
Per core: QKV column-parallel, per-head attention (scores computed transposed,
softmax sums via a ones-column appended to V in the PV matmul, max-subtraction
skipped -- scores are O(5) so exp is safe), then a pairwise AllGather of the
normalized per-head outputs and a column-parallel output projection.

All matmul operands are bf16 (fp32 PSUM accumulation); fp32 matmul on trn2
costs two array passes, bf16 one. The host pre-transposes x to x^T [D, S]
and casts to bf16 (input prep), so no on-device transpose is needed.

QKV projection (per head-pair) and attention are emitted interleaved so the
scalar-engine exp stream (the critical resource) starts as early as possible
and PE fills its gaps with the next head-pair's projections. The AllGather
is split per head-pair so all but the last overlap attention.

Host assembly: out[b] = concat(core 2b cols 0:512, core 2b+1 cols 512:1024).

Self-contained: hardcodes all shapes; builds/compiles once per process.
"""

from contextlib import ExitStack

import ml_dtypes
import numpy as np

import concourse.bass as bass
import concourse.mybir as mybir
import concourse.tile as tile
from concourse import bacc
from concourse.bass_utils import run_bass_kernel_spmd

FP = mybir.dt.float32
BF = mybir.dt.bfloat16
AFT = mybir.ActivationFunctionType
ts = bass.ts

NCORES = 8
D = 1024           # d_model
HD = 64            # head dim
HPC = 8            # heads per core
DQ = HPC * HD      # per-core qkv width = 512
SCALE = 1.0 / np.sqrt(HD)


def emit_mha(nc, tc, io, S, dbg=False):
    """Emit the per-core MHA program. io: dict of DRAM APs."""
    NHP = HPC // 2       # head pairs = 4
    KT = S // 128        # sk tiles
    SQB = S // 512       # sq blocks of 512
    DKT = D // 128       # d_in tiles = 8
    MQ = DQ // 128       # qkv dout tiles = 4
    TT = S // 128        # token tiles
    NB = S // 512        # token blocks of 512

    xt_in, wq_in, bqk_in, wk_in, wv_in, bv_in, wo_in, bo_in, out_ext = (
        io["xt"], io["wq"], io["bqk"], io["wk"], io["wv"], io["bv"],
        io["wo"], io["bo"], io["out"])

    with ExitStack() as ctx:
        const_pool = ctx.enter_context(tc.tile_pool(name="const", bufs=1))
        dram_pool = ctx.enter_context(tc.tile_pool(name="dram", bufs=1, space="DRAM"))
        # one shared PSUM budget: mm 2 + scores 4 + accA 1 + accB 1 = 8 banks
        mm_psum = ctx.enter_context(
            tc.tile_pool(name="mmps", bufs=2, space="PSUM"))
        sc_psum = ctx.enter_context(
            tc.tile_pool(name="scps", bufs=2, space="PSUM"))
        ac_psum = ctx.enter_context(
            tc.tile_pool(name="acps", bufs=1, space="PSUM"))

        # biases for q/k, host-packed [128, 2*MQ]: col m = bq tile m, MQ+m = bk
        bias_qk = const_pool.tile([128, 2 * MQ], FP, tag="bqk", name="bqk")
        nc.sync.dma_start(bias_qk[:], bqk_in[:, :])

        # bv broadcast tile [128, DQ]
        bv_bc = const_pool.tile([128, DQ], FP, tag="bvbc", name="bvbc")
        with tc.tile_pool(name="btmpp", bufs=1) as btmp_pool:
            btmp = btmp_pool.tile([128, DQ], FP, tag="btmp", name="btmp")
            nc.sync.dma_start(
                btmp[0:1, :], bv_in[:].rearrange("(one f) -> one f", one=1))
            nc.gpsimd.partition_broadcast(bv_bc[:], btmp[0:1, :])

        def sum_slot(h, sqb):
            # unit (h, sqb) -> partition 32*(h%4), cols [(h//4)*SQB + sqb]*512
            return 32 * (h % 4), ts((h // 4) * SQB + sqb, 512)

        # DRAM bounce + per-(head-pair, seq-half) AllGather in/out (bf16);
        # collective operands must be contiguous
        y_bnc = [[dram_pool.tile([128, S // 2], BF, tag=f"ybounce{hp}_{h2}",
                                 name=f"ybounce{hp}_{h2}")
                  for h2 in range(2)]
                 for hp in range(NHP)]
        y_gath = [[dram_pool.tile([256, S // 2], BF, tag=f"ygather{hp}_{h2}",
                                  name=f"ygather{hp}_{h2}")
                   for h2 in range(2)]
                  for hp in range(NHP)]

        with ExitStack() as phase12:
            qkv_pool = phase12.enter_context(tc.tile_pool(name="qkv", bufs=1))
            yt_pool = phase12.enter_context(tc.tile_pool(name="yt", bufs=1))
            exp_pool = phase12.enter_context(tc.tile_pool(name="exp", bufs=6))
            stage_pool = phase12.enter_context(tc.tile_pool(name="stage", bufs=3))

            # q^T / k^T, d-major: tile hp holds heads 2hp (parts 0-63), 2hp+1
            qT = [qkv_pool.tile([128, S], BF, tag=f"qT{m}", name=f"qT{m}")
                  for m in range(MQ)]
            kT = [qkv_pool.tile([128, S], BF, tag=f"kT{m}", name=f"kT{m}")
                  for m in range(MQ)]
            # v natural [tok, dout] with a ones column per head
            v_ones = [qkv_pool.tile([128, HPC * (HD + 1)], BF, tag=f"v{t}",
                                    name=f"v{t}")
                      for t in range(TT)]
            # y^T (attention out, d-major, normalized in place per unit)
            yT = [yt_pool.tile([128, S], BF, tag=f"yT{m}", name=f"yT{m}")
                  for m in range(MQ)]
            # softmax sums / reciprocals, packed 32-partition-aligned
            sums_t = yt_pool.tile([128, 2 * SQB * 512], FP, tag="sums",
                                  name="sums")
            recip_t = yt_pool.tile([128, 2 * SQB * 512], FP, tag="recip",
                                   name="recip")
            nc.gpsimd.memset(sums_t[:], 1.0)

            with ExitStack() as phase01:
                # ---- load x^T (pre-transposed on host) and weights ----
                xtw_pool = phase01.enter_context(tc.tile_pool(name="xtw", bufs=1))
                xTall = xtw_pool.tile([128, DKT * S], BF, tag="xTall",
                                      name="xTall")
                xT3 = xTall[:].rearrange("p (d s) -> p d s", s=S)

                def xTs(k, sl):
                    return xT3[:, k, sl]

                # full row-blocks [128, DQ] -> large contiguous descriptors
                wv_t = [xtw_pool.tile([128, DQ], BF, tag=f"wv{k}",
                                      name=f"wv{k}")
                        for k in range(DKT)]
                wq_t = [xtw_pool.tile([128, DQ], BF, tag=f"wq{k}",
                                      name=f"wq{k}")
                        for k in range(DKT)]
                wk_t = [xtw_pool.tile([128, DQ], BF, tag=f"wk{k}",
                                      name=f"wk{k}")
                        for k in range(DKT)]
                # load order follows first consumption: wq + x chunk 0 feed
                # the first projection group; x is loaded in 512-col chunks
                # so the work spreads across DMA queues
                for k in range(DKT):
                    nc.sync.dma_start(wq_t[k][:], wq_in[ts(k, 128), :])
                for k in range(DKT):
                    nc.sync.dma_start(xT3[:, k, ts(0, 512)],
                                      xt_in[ts(k, 128), ts(0, 512)])
                for k in range(DKT):
                    nc.sync.dma_start(wk_t[k][:], wk_in[ts(k, 128), :])
                for k in range(DKT):
                    nc.sync.dma_start(wv_t[k][:], wv_in[ts(k, 128), :])
                for nb in range(1, NB):
                    for k in range(DKT):
                        nc.sync.dma_start(xT3[:, k, ts(nb, 512)],
                                          xt_in[ts(k, 128), ts(nb, 512)])

                def emit_qk_group(m, g):
                    # one q/k projection psum group for head-pair m;
                    # g//NB selects q vs k, g%NB the token block. Evac on
                    # DVE (keeps the scalar engine free for the exp stream).
                    w_t, bcol, dstT = ((wq_t, 0, qT), (wk_t, 1, kT))[g // NB]
                    nb = g % NB
                    ps = mm_psum.tile([128, 512], FP, tag="mm", name="mm")
                    for k in range(DKT):
                        nc.tensor.matmul(
                            ps[:], lhsT=w_t[k][:, ts(m, 128)],
                            rhs=xTs(k, ts(nb, 512)),
                            start=(k == 0), stop=(k == DKT - 1))
                    col = bcol * MQ + m
                    nc.vector.tensor_scalar_add(
                        dstT[m][:, ts(nb, 512)], ps[:],
                        bias_qk[:, col:col + 1])

                def emit_v(t0, t1):
                    for ti in range(t0, t1):
                        ps = mm_psum.tile([128, DQ], FP, tag="mm", name="mm")
                        for k in range(DKT):
                            nc.tensor.matmul(
                                ps[:], lhsT=xTs(k, ts(ti, 128)), rhs=wv_t[k][:],
                                start=(k == 0), stop=(k == DKT - 1))
                        vt3 = v_ones[ti][:].rearrange("p (h u) -> p h u",
                                                      u=HD + 1)
                        nc.vector.tensor_add(
                            vt3[:, :, 0:HD],
                            ps[:].rearrange("p (h u) -> p h u", u=HD),
                            bv_bc[:].rearrange("p (h u) -> p h u", u=HD))
                        nc.gpsimd.memset(vt3[:, :, HD:HD + 1], 1.0)

                # ---- per head-pair: qk projection then attention; v right
                # after hp0's qk so the exp stream starts asap ----
                for hp in range(NHP):
                    if hp == 0:
                        for g in range(2 * NB):
                            emit_qk_group(0, g)

                    hA, hB = 2 * hp, 2 * hp + 1
                    for sqb in range(SQB):
                        sq = ts(sqb, 512)
                        accA = ac_psum.tile([HD + 1, 512], FP, tag="accA",
                                            name="accA")
                        accB = ac_psum.tile([HD + 1, 512], FP, tag="accB",
                                            name="accB")
                        for k in range(KT):
                            sk = ts(k, 128)
                            ps = sc_psum.tile([128, 1024], FP, tag="sc", name="sc")
                            # scores^T [sk, sq] for both heads (row-tiled pair)
                            nc.tensor.matmul(
                                ps[:, 0:512], lhsT=kT[hp][0:64, sk],
                                rhs=qT[hp][0:64, sq], start=True, stop=True)
                            nc.tensor.matmul(
                                ps[:, 512:1024], lhsT=kT[hp][64:128, sk],
                                rhs=qT[hp][64:128, sq], start=True, stop=True)
                            et = exp_pool.tile([128, 1024], BF, tag="exp",
                                               name="exp")
                            nc.scalar.activation(et[:], ps[:], AFT.Exp,
                                                 scale=SCALE)
                            if hp == 0 and sqb == 0:
                                # produce v[k] just in time for its attnv
                                emit_v(k, k + 1)
                            # y^T accumulation: lhsT = [v_h | 1]
                            nc.tensor.matmul(
                                accA[:], lhsT=v_ones[k][:, hA * 65:hA * 65 + 65],
                                rhs=et[:, 0:512],
                                start=(k == 0), stop=(k == KT - 1),
                                skip_group_check=True)
                            nc.tensor.matmul(
                                accB[:], lhsT=v_ones[k][:, hB * 65:hB * 65 + 65],
                                rhs=et[:, 512:1024],
                                start=(k == 0), stop=(k == KT - 1),
                                skip_group_check=True)
                        # extract y (rows 0-63) and sums (row 64)
                        nc.vector.tensor_copy(yT[hp][0:64, sq], accA[0:64, :])
                        st = stage_pool.tile([128, 512], BF, tag="bst", name="bst")
                        nc.vector.tensor_copy(st[0:64, :], accB[0:64, :])
                        nc.sync.dma_start(yT[hp][64:128, sq], st[0:64, :])
                        for acc, h in ((accA, hA), (accB, hB)):
                            sp, sc = sum_slot(h, sqb)
                            sA = stage_pool.tile([128, 512], FP, tag="sst",
                                                 name="sst")
                            nc.vector.tensor_copy(sA[64:65, :], acc[64:65, :])
                            nc.sync.dma_start(sums_t[sp:sp + 1, sc],
                                              sA[64:65, :])
                        # reciprocal for this unit (both heads share a
                        # 64-partition band and column slot)
                        band = 32 * (hA % 4)
                        _, sc = sum_slot(hA, sqb)
                        nc.vector.reciprocal(
                            recip_t[band:band + 64, sc],
                            sums_t[band:band + 64, sc])
                        # normalize y^T for this unit in place
                        for h2, h in ((0, hA), (1, hB)):
                            rows = slice(64 * h2, 64 * h2 + 64)
                            sp, _ = sum_slot(h, sqb)
                            # HW partition_broadcast reads partition 0 of the
                            # tensor regardless of the AP base -> stage the
                            # recip row to partition 0 (cast to bf16) first.
                            rtmp = stage_pool.tile([128, 512], FP, tag="rtmp",
                                                   name="rtmp")
                            nc.sync.dma_start(rtmp[0:1, :],
                                              recip_t[sp:sp + 1, sc])
                            rtb = stage_pool.tile([128, 512], BF, tag="rtb",
                                                  name="rtb")
                            nc.vector.tensor_copy(rtb[0:1, :], rtmp[0:1, :])
                            rb = stage_pool.tile([128, 512], BF, tag="rb",
                                                 name="rb")
                            nc.gpsimd.partition_broadcast(rb[:], rtb[0:1, :])
                            nc.vector.tensor_mul(
                                yT[hp][rows, sq], yT[hp][rows, sq],
                                rb[rows, :])
                        # drip the next head-pair's projections between units
                        if hp + 1 < NHP:
                            gpu = 2 * NB // SQB
                            for g in range(gpu * sqb, gpu * (sqb + 1)):
                                emit_qk_group(hp + 1, g)
                        # ship and AllGather each completed seq half of this
                        # hp's y^T (overlaps remaining compute)
                        covered = (sqb + 1) * 512
                        for h2 in range(2):
                            end = (h2 + 1) * (S // 2)
                            if covered >= end > covered - 512:
                                half = ts(h2, S // 2)
                                nc.sync.dma_start(y_bnc[hp][h2][:, :],
                                                  yT[hp][:, half])
                                nc.gpsimd.collective_compute(
                                    "AllGather", mybir.AluOpType.bypass,
                                    replica_groups=[[0, 1], [2, 3],
                                                    [4, 5], [6, 7]],
                                    ins=[y_bnc[hp][h2][:, :]],
                                    outs=[y_gath[hp][h2][:, :]])
        # qkv/yt/xtw pools freed here

        if dbg:
            for hp in range(NHP):
                for h2 in range(2):
                    nc.sync.dma_start(io["dbg_y"][ts(hp, 128), ts(h2, S // 2)],
                                      y_bnc[hp][h2][:, :])
            for hp in range(NHP):
                for h2 in range(2):
                    half = ts(h2, S // 2)
                    nc.sync.dma_start(io["dbg_yg"][ts(hp, 128), half],
                                      y_gath[hp][h2][0:128, :])
                    nc.sync.dma_start(
                        io["dbg_yg"][DQ + 128 * hp:DQ + 128 * hp + 128, half],
                        y_gath[hp][h2][128:256, :])

        # ---- Phase 4: out = y_full @ Wo[:, cols] + bo ----
        with tc.tile_pool(name="p4", bufs=1) as p4_pool, \
             tc.tile_pool(name="outp", bufs=3) as out_pool:
            bo_bc = p4_pool.tile([128, DQ], FP, tag="bobc", name="bobc")
            btmp2 = p4_pool.tile([128, DQ], FP, tag="btmp2", name="btmp2")
            nc.sync.dma_start(
                btmp2[0:1, :], bo_in[:].rearrange("(one f) -> one f", one=1))
            nc.gpsimd.partition_broadcast(bo_bc[:], btmp2[0:1, :])
            yg = [p4_pool.tile([128, S], BF, tag=f"yg{k}", name=f"yg{k}")
                  for k in range(2 * MQ)]
            wot = [p4_pool.tile([128, DQ], BF, tag=f"wo{k}", name=f"wo{k}")
                   for k in range(2 * MQ)]
            for k2 in range(2 * MQ):
                nc.sync.dma_start(wot[k2][:], wo_in[ts(k2, 128), :])
            for hp in range(NHP):
                # rank-0 rows = global ydim 128*hp, rank-1 rows = 512 + 128*hp
                for h2 in range(2):
                    half = ts(h2, S // 2)
                    nc.sync.dma_start(yg[hp][:, half], y_gath[hp][h2][0:128, :])
                    nc.sync.dma_start(yg[MQ + hp][:, half],
                                      y_gath[hp][h2][128:256, :])
            for ti in range(TT):
                po = mm_psum.tile([128, DQ], FP, tag="mm", name="mm")
                for k2 in range(2 * MQ):
                    nc.tensor.matmul(
                        po[:], lhsT=yg[k2][:, ts(ti, 128)], rhs=wot[k2][:],
                        start=(k2 == 0), stop=(k2 == 2 * MQ - 1))
                ot = out_pool.tile([128, DQ], FP, tag="ot", name="ot")
                nc.vector.tensor_add(ot[:], po[:], bo_bc[:])
                nc.sync.dma_start(out_ext[ts(ti, 128), :], ot[:])


def build_program(S=2048, dbg=False):
    nc = bacc.Bacc(
        "TRN2",
        target_bir_lowering=False,
        debug=False,
        enable_asserts=True,
        num_devices=NCORES,
    )
    io = {
        "xt": nc.declare_dram_parameter("xt", [D, S], BF, isOutput=False),
        "wq": nc.declare_dram_parameter("wq", [D, DQ], BF, isOutput=False),
        "bqk": nc.declare_dram_parameter("bqk", [128, 8], FP, isOutput=False),
        "wk": nc.declare_dram_parameter("wk", [D, DQ], BF, isOutput=False),
        "wv": nc.declare_dram_parameter("wv", [D, DQ], BF, isOutput=False),
        "bv": nc.declare_dram_parameter("bv", [DQ], FP, isOutput=False),
        "wo": nc.declare_dram_parameter("wo", [D, DQ], BF, isOutput=False),
        "bo": nc.declare_dram_parameter("bo", [DQ], FP, isOutput=False),
        "out": nc.declare_dram_parameter("out", [S, DQ], FP, isOutput=True),
    }
    if dbg:
        io["dbg_y"] = nc.declare_dram_parameter(
            "dbg_y", [DQ, S], BF, isOutput=True)
        io["dbg_yg"] = nc.declare_dram_parameter(
            "dbg_yg", [2 * DQ, S], BF, isOutput=True)
    io = {k: (v[:] if not isinstance(v, bass.AP) else v) for k, v in io.items()}
    with tile.TileContext(nc) as tc:
        emit_mha(nc, tc, io, S, dbg=dbg)
    nc.finalize()
    return nc


def shard_inputs(x, Wq, bq, Wk, bk, Wv, bv, Wo, bo):
    """Full inputs -> per-core in_maps. Matmul operands cast to bf16; x is
    transposed on the host (input prep for the d-major device layout)."""
    BFNP = ml_dtypes.bfloat16
    f32 = lambda a: np.ascontiguousarray(np.asarray(a), dtype=np.float32)
    bf = lambda a: np.ascontiguousarray(np.asarray(a, dtype=np.float32)
                                        .astype(BFNP))
    x = np.asarray(x, dtype=np.float32).astype(BFNP)
    xts = [np.ascontiguousarray(x[b].T) for b in range(4)]
    Wq, Wk, Wv, Wo = bf(Wq), bf(Wk), bf(Wv), bf(Wo)
    bq, bk, bv, bo = f32(bq), f32(bk), f32(bv), f32(bo)
    in_maps = []
    for c in range(NCORES):
        b, g = divmod(c, 2)
        sl = slice(g * DQ, (g + 1) * DQ)
        bqk = np.empty((128, 8), np.float32)
        for m in range(4):
            bqk[:, m] = bq[sl][m * 128:(m + 1) * 128]
            bqk[:, 4 + m] = bk[sl][m * 128:(m + 1) * 128]
        in_maps.append({
            "xt": xts[b],
            "wq": np.ascontiguousarray(Wq[:, sl]), "bqk": bqk,
            "wk": np.ascontiguousarray(Wk[:, sl]),
            "wv": np.ascontiguousarray(Wv[:, sl]), "bv": bv[sl].copy(),
            "wo": np.ascontiguousarray(Wo[:, sl]), "bo": bo[sl].copy(),
        })
    return in_maps


_CACHE = {}


def _get_program(S=2048):
    if S not in _CACHE:
        _CACHE[S] = build_program(S)
    return _CACHE[S]


def kernel(x, Wq, bq, Wk, bk, Wv, bv, Wo, bo):
    nc = _get_program(2048)
    in_maps = shard_inputs(x, Wq, bq, Wk, bk, Wv, bv, Wo, bo)
    res = run_bass_kernel_spmd(nc, in_maps, list(range(NCORES))).results
    S = 2048
    out = np.empty((4, S, D), dtype=np.float32)
    for c in range(NCORES):
        b, g = divmod(c, 2)
        out[b, :, g * DQ:(g + 1) * DQ] = res[c]["out"]
    return out



# revision 6
# speedup vs baseline: 1.0262x; 1.0262x over previous
"""Trainium2 Bass kernel: multi-head attention (b=4, s=2048, d_model=1024, h=16).

Sharding over 8 NeuronCores: 2-D (batch x head-half).
  core c -> batch c//2, head group c%2 (8 of 16 heads, qkv dims 512*g..512*g+512).
Per core: QKV column-parallel, per-head attention (scores computed transposed,
softmax sums via a ones-column appended to V in the PV matmul, max-subtraction
skipped -- scores are O(5) so exp is safe), then a per-(head-pair, seq-quarter)
AllGather of the normalized per-head outputs and a column-parallel output
projection.

The scalar-engine exp stream (~285us) is the critical path; the kernel is
organized so it starts ASAP and never stalls:
  - DMA order: wk m0 slices + x chunk0 + wq m0 first -> first exp ~12us.
  - Only the two projection groups the first attention unit needs are emitted
    upfront; all other QKV projection groups are dripped as whole psum-groups
    at paced drop points inside the attention units.
  - y is shipped + AllGathered per (head-pair, 512-token quarter); the output
    projection for quarters 0-2 is dripped into the last head-pair's units so
    only quarter 3 (+ its gather) remains after the last exp.
  - x/weight SBUF is freed when the last projection drips finish (start of
    hp3) and reused for Wo / gathered-y tiles.

All matmul operands are bf16 (fp32 PSUM accumulation). The host pre-transposes
x to x^T [D, S] and casts to bf16, so no on-device transpose is needed. The
two score matmuls per (head-pair, k) are row-tiled (tile_position auto-derived
from base partitions 0/64) and run concurrently on the PE.

Host assembly: out[b] = concat(core 2b cols 0:512, core 2b+1 cols 512:1024).

Self-contained: hardcodes all shapes; builds/compiles once per process.
"""

from contextlib import ExitStack

import ml_dtypes
import numpy as np

import concourse.bass as bass
import concourse.mybir as mybir
import concourse.tile as tile
from concourse import bacc
from concourse.bass_utils import run_bass_kernel_spmd

FP = mybir.dt.float32
BF = mybir.dt.bfloat16
AFT = mybir.ActivationFunctionType
ts = bass.ts

NCORES = 8
D = 1024           # d_model
HD = 64            # head dim
HPC = 8            # heads per core
DQ = HPC * HD      # per-core qkv width = 512
SCALE = 1.0 / np.sqrt(HD)


def emit_mha(nc, tc, io, S):
    """Emit the per-core MHA program. io: dict of DRAM APs."""
    NHP = HPC // 2       # head pairs = 4
    KT = S // 128        # sk tiles = 16
    SQB = S // 512       # sq blocks of 512 = 4
    DKT = D // 128       # d_in tiles = 8
    MQ = DQ // 128       # qkv dout tiles = 4
    TT = S // 128        # token tiles = 16
    NB = S // 512        # token blocks of 512 = 4

    xt_in, wq_in, bqk_in, wk_in, wv_in, bv_in, wo_in, bo_in, out_ext = (
        io["xt"], io["wq"], io["bqk"], io["wk"], io["wv"], io["bv"],
        io["wo"], io["bo"], io["out"])

    with ExitStack() as ctx:
        const_pool = ctx.enter_context(tc.tile_pool(name="const", bufs=1))
        dram_pool = ctx.enter_context(tc.tile_pool(name="dram", bufs=1, space="DRAM"))
        # PSUM budget: mm 2 + scores 2x2 + accA 1 + accB 1 = 8 banks
        mm_psum = ctx.enter_context(
            tc.tile_pool(name="mmps", bufs=2, space="PSUM"))
        sc_psum = ctx.enter_context(
            tc.tile_pool(name="scps", bufs=2, space="PSUM"))
        ac_psum = ctx.enter_context(
            tc.tile_pool(name="acps", bufs=1, space="PSUM"))

        # biases for q/k, host-packed [128, 2*MQ]: col m = bq tile m, MQ+m = bk
        bias_qk = const_pool.tile([128, 2 * MQ], FP, tag="bqk", name="bqk")
        nc.sync.dma_start(bias_qk[:], bqk_in[:, :])

        # bv broadcast tile [128, DQ]
        bv_bc = const_pool.tile([128, DQ], FP, tag="bvbc", name="bvbc")
        # bo broadcast tile [128, DQ] (built now; used by the out projection)
        bo_bc = const_pool.tile([128, DQ], FP, tag="bobc", name="bobc")
        with tc.tile_pool(name="btmpp", bufs=2) as btmp_pool:
            btmp = btmp_pool.tile([128, DQ], FP, tag="btmp", name="btmp")
            nc.sync.dma_start(
                btmp[0:1, :], bv_in[:].rearrange("(one f) -> one f", one=1))
            nc.gpsimd.partition_broadcast(bv_bc[:], btmp[0:1, :])
            btmp2 = btmp_pool.tile([128, DQ], FP, tag="btmp", name="btmp2")
            nc.sync.dma_start(
                btmp2[0:1, :], bo_in[:].rearrange("(one f) -> one f", one=1))
            nc.gpsimd.partition_broadcast(bo_bc[:], btmp2[0:1, :])

        def sum_slot(h, sqb):
            # unit (h, sqb) -> partition 32*(h%4), cols [(h//4)*SQB + sqb]*512
            return 32 * (h % 4), ts((h // 4) * SQB + sqb, 512)

        # DRAM bounce + per-(head-pair, seq-quarter) AllGather in/out (bf16);
        # collective operands must be contiguous
        y_bnc = [[dram_pool.tile([128, 512], BF, tag=f"ybounce{hp}_{q}",
                                 name=f"ybounce{hp}_{q}")
                  for q in range(SQB)]
                 for hp in range(NHP)]
        y_gath = [[dram_pool.tile([256, 512], BF, tag=f"ygather{hp}_{q}",
                                  name=f"ygather{hp}_{q}")
                   for q in range(SQB)]
                  for hp in range(NHP)]

        with ExitStack() as phase12:
            qkv_pool = phase12.enter_context(tc.tile_pool(name="qkv", bufs=1))
            yt_pool = phase12.enter_context(tc.tile_pool(name="yt", bufs=1))
            exp_pool = phase12.enter_context(tc.tile_pool(name="exp", bufs=6))
            stage_pool = phase12.enter_context(tc.tile_pool(name="stage", bufs=3))

            # q^T / k^T, d-major: tile hp holds heads 2hp (parts 0-63), 2hp+1
            qT = [qkv_pool.tile([128, S], BF, tag=f"qT{m}", name=f"qT{m}")
                  for m in range(MQ)]
            kT = [qkv_pool.tile([128, S], BF, tag=f"kT{m}", name=f"kT{m}")
                  for m in range(MQ)]
            # v natural [tok, dout] with a ones column per head
            v_ones = [qkv_pool.tile([128, HPC * (HD + 1)], BF, tag=f"v{t}",
                                    name=f"v{t}")
                      for t in range(TT)]
            # y^T (attention out, d-major, normalized in place per unit)
            yT = [yt_pool.tile([128, S], BF, tag=f"yT{m}", name=f"yT{m}")
                  for m in range(MQ)]
            # softmax sums / reciprocals, packed 32-partition-aligned
            sums_t = yt_pool.tile([128, 2 * SQB * 512], FP, tag="sums",
                                  name="sums")
            recip_t = yt_pool.tile([128, 2 * SQB * 512], FP, tag="recip",
                                   name="recip")
            nc.gpsimd.memset(sums_t[:], 1.0)

            phase01 = ExitStack()
            xtw_pool = phase01.enter_context(tc.tile_pool(name="xtw", bufs=1))
            xTall = xtw_pool.tile([128, DKT * S], BF, tag="xTall",
                                  name="xTall")
            xT3 = xTall[:].rearrange("p (d s) -> p d s", s=S)

            def xTs(k, sl):
                return xT3[:, k, sl]

            # wq/wk as per-(k, hp) [128, 128] column slices so the DMA order
            # can prioritize exactly what the first attention unit needs
            wq_s = [[xtw_pool.tile([128, 128], BF, tag=f"wq{k}_{m}",
                                   name=f"wq{k}_{m}")
                     for m in range(MQ)] for k in range(DKT)]
            wk_s = [[xtw_pool.tile([128, 128], BF, tag=f"wk{k}_{m}",
                                   name=f"wk{k}_{m}")
                     for m in range(MQ)] for k in range(DKT)]
            wv_t = [xtw_pool.tile([128, DQ], BF, tag=f"wv{k}", name=f"wv{k}")
                    for k in range(DKT)]

            # ---- DMA order = first-consumption order ----
            for k in range(DKT):
                nc.sync.dma_start(wk_s[k][0][:], wk_in[ts(k, 128), ts(0, 128)])
            for k in range(DKT):
                nc.sync.dma_start(xT3[:, k, ts(0, 512)],
                                  xt_in[ts(k, 128), ts(0, 512)])
            for k in range(DKT):
                nc.sync.dma_start(wq_s[k][0][:], wq_in[ts(k, 128), ts(0, 128)])
            for k in range(DKT):
                nc.sync.dma_start(wv_t[k][:], wv_in[ts(k, 128), :])
            for nb in range(1, NB):
                for k in range(DKT):
                    nc.sync.dma_start(xT3[:, k, ts(nb, 512)],
                                      xt_in[ts(k, 128), ts(nb, 512)])
            for m in range(1, MQ):
                for k in range(DKT):
                    nc.sync.dma_start(wk_s[k][m][:],
                                      wk_in[ts(k, 128), ts(m, 128)])
                for k in range(DKT):
                    nc.sync.dma_start(wq_s[k][m][:],
                                      wq_in[ts(k, 128), ts(m, 128)])

            def emit_qk_group(hp, which, nb):
                # one q/k projection psum group for head-pair hp; which: 0=q,
                # 1=k; nb = token block. Evac on DVE (keeps the scalar engine
                # free for the exp stream).
                w_s, dstT = ((wq_s, qT), (wk_s, kT))[which]
                ps = mm_psum.tile([128, 512], FP, tag="mm", name="mm")
                for k in range(DKT):
                    nc.tensor.matmul(
                        ps[:], lhsT=w_s[k][hp][:], rhs=xTs(k, ts(nb, 512)),
                        start=(k == 0), stop=(k == DKT - 1))
                col = which * MQ + hp
                nc.vector.tensor_scalar_add(
                    dstT[hp][:, ts(nb, 512)], ps[:],
                    bias_qk[:, col:col + 1])

            def emit_v(ti):
                ps = mm_psum.tile([128, DQ], FP, tag="mm", name="mm")
                for k in range(DKT):
                    nc.tensor.matmul(
                        ps[:], lhsT=xTs(k, ts(ti, 128)), rhs=wv_t[k][:],
                        start=(k == 0), stop=(k == DKT - 1))
                vt3 = v_ones[ti][:].rearrange("p (h u) -> p h u", u=HD + 1)
                nc.vector.tensor_add(
                    vt3[:, :, 0:HD],
                    ps[:].rearrange("p (h u) -> p h u", u=HD),
                    bv_bc[:].rearrange("p (h u) -> p h u", u=HD))
                nc.gpsimd.memset(vt3[:, :, HD:HD + 1], 1.0)

            # ---- upfront: the two groups unit (0,0) needs to start ----
            emit_qk_group(0, 1, 0)   # k proj, head-pair 0, tokens 0:512
            emit_qk_group(0, 0, 0)   # q proj, head-pair 0, tokens 0:512

            # ---- paced projection-drip schedule: {(hp,sqb): {iter: [fn]}} ----
            drip = {}

            def add_drop(hp, sqb, it, fn):
                drip.setdefault((hp, sqb), {}).setdefault(it, []).append(fn)

            # remaining hp0 K groups + Q nb1 inside unit (0,0), ahead of use
            for i, (w, nb) in enumerate(((1, 1), (1, 2), (1, 3), (0, 1))):
                add_drop(0, 0, 3 * i + 1,
                         lambda w=w, nb=nb: emit_qk_group(0, w, nb))
            # hp0's late Q groups + hp1's 8 groups over hp0's units 1-3
            sched0 = [
                (1, (0, 0, 2), (1, 1, 0), (1, 1, 1), (1, 1, 2)),
                (2, (0, 0, 3), (1, 1, 3), (1, 0, 0), (1, 0, 1)),
                (3, (1, 0, 2), (1, 0, 3)),
            ]
            for sqb, *groups in sched0:
                for i, (ghp, w, nb) in enumerate(groups):
                    add_drop(0, sqb, 4 * i + 1,
                             lambda ghp=ghp, w=w, nb=nb:
                                 emit_qk_group(ghp, w, nb))
            # next head-pair's 8 groups over this head-pair's units 1-3
            for hp in range(1, NHP - 1):
                seq = [(1, nb) for nb in range(NB)] + \
                      [(0, nb) for nb in range(NB)]
                for i, (w, nb) in enumerate(seq):
                    sqb, it = 1 + i // 3, 5 * (i % 3) + 1
                    add_drop(hp, sqb, it,
                             lambda hp=hp, w=w, nb=nb:
                                 emit_qk_group(hp + 1, w, nb))

            # ---- out-projection: per seq-quarter, dripped into hp3 units ----
            wot = [None] * (2 * MQ)
            ygq_pool_ref = [None]
            out_stage_ref = [None]

            def load_wo():
                p4_pool = phase12.enter_context(
                    tc.tile_pool(name="p4", bufs=1))
                ygq_pool_ref[0] = phase12.enter_context(
                    tc.tile_pool(name="ygq", bufs=2))
                out_stage_ref[0] = phase12.enter_context(
                    tc.tile_pool(name="outp", bufs=3))
                for k2 in range(2 * MQ):
                    wot[k2] = p4_pool.tile([128, DQ], BF, tag=f"wo{k2}",
                                           name=f"wo{k2}")
                    nc.sync.dma_start(wot[k2][:], wo_in[ts(k2, 128), :])

            def start_quarter(q):
                # load gathered y for quarter q: rows k2=hp from rank0,
                # k2=MQ+hp from rank1 of each pair gather
                ygq = [ygq_pool_ref[0].tile([128, 512], BF, tag=f"yg{k2}",
                                            name=f"yg{k2}_{q}")
                       for k2 in range(2 * MQ)]
                for hp2 in range(NHP):
                    nc.sync.dma_start(ygq[hp2][:], y_gath[hp2][q][0:128, :])
                    nc.sync.dma_start(ygq[MQ + hp2][:],
                                      y_gath[hp2][q][128:256, :])
                return ygq

            def emit_outproj_ti(ygq, q, tl):
                # one 128-token tile of the output projection
                po = mm_psum.tile([128, DQ], FP, tag="mm", name="mm")
                for k2 in range(2 * MQ):
                    nc.tensor.matmul(
                        po[:], lhsT=ygq[k2][:, ts(tl, 128)], rhs=wot[k2][:],
                        start=(k2 == 0), stop=(k2 == 2 * MQ - 1))
                ot = out_stage_ref[0].tile([128, DQ], FP, tag="ot", name="ot")
                nc.vector.tensor_add(ot[:], po[:], bo_bc[:])
                nc.sync.dma_start(out_ext[ts(4 * q + tl, 128), :], ot[:])

            ygq_live = {}

            for q in range(SQB - 1):
                def qstart(q=q):
                    ygq_live[q] = start_quarter(q)
                add_drop(3, q + 1, 0, qstart)
                for tl in range(4):
                    add_drop(3, q + 1, 4 * tl + 2,
                             lambda q=q, tl=tl:
                                 emit_outproj_ti(ygq_live[q], q, tl))

            # ---- attention units ----
            for hp in range(NHP):
                if hp == NHP - 1:
                    # all projection drips done -> free x/weights, bring in Wo
                    phase01.close()
                    load_wo()

                hA, hB = 2 * hp, 2 * hp + 1
                for sqb in range(SQB):
                    sq = ts(sqb, 512)
                    drops = drip.get((hp, sqb), {})
                    accA = ac_psum.tile([HD + 1, 512], FP, tag="accA",
                                        name="accA")
                    accB = ac_psum.tile([HD + 1, 512], FP, tag="accB",
                                        name="accB")
                    for k in range(KT):
                        sk = ts(k, 128)
                        ps = sc_psum.tile([128, 1024], FP, tag="sc", name="sc")
                        # scores^T [sk, sq] for both heads; base partitions
                        # 0/64 -> row-tiled, the two matmuls run concurrently
                        nc.tensor.matmul(
                            ps[:, 0:512], lhsT=kT[hp][0:64, sk],
                            rhs=qT[hp][0:64, sq], start=True, stop=True)
                        nc.tensor.matmul(
                            ps[:, 512:1024], lhsT=kT[hp][64:128, sk],
                            rhs=qT[hp][64:128, sq], start=True, stop=True)
                        et = exp_pool.tile([128, 1024], BF, tag="exp",
                                           name="exp")
                        nc.scalar.activation(et[:], ps[:], AFT.Exp,
                                             scale=SCALE)
                        if hp == 0 and sqb == 0:
                            # produce v[k] just in time for its attnv
                            emit_v(k)
                        # y^T accumulation: lhsT = [v_h | 1]
                        nc.tensor.matmul(
                            accA[:], lhsT=v_ones[k][:, hA * 65:hA * 65 + 65],
                            rhs=et[:, 0:512],
                            start=(k == 0), stop=(k == KT - 1),
                            skip_group_check=True)
                        nc.tensor.matmul(
                            accB[:], lhsT=v_ones[k][:, hB * 65:hB * 65 + 65],
                            rhs=et[:, 512:1024],
                            start=(k == 0), stop=(k == KT - 1),
                            skip_group_check=True)
                        for fn in drops.get(k, ()):
                            fn()
                    # extract y (rows 0-63) and sums (row 64)
                    nc.vector.tensor_copy(yT[hp][0:64, sq], accA[0:64, :])
                    st = stage_pool.tile([128, 512], BF, tag="bst", name="bst")
                    nc.vector.tensor_copy(st[0:64, :], accB[0:64, :])
                    nc.sync.dma_start(yT[hp][64:128, sq], st[0:64, :])
                    for acc, h in ((accA, hA), (accB, hB)):
                        sp, sc = sum_slot(h, sqb)
                        sA = stage_pool.tile([128, 512], FP, tag="sst",
                                             name="sst")
                        nc.vector.tensor_copy(sA[64:65, :], acc[64:65, :])
                        nc.sync.dma_start(sums_t[sp:sp + 1, sc],
                                          sA[64:65, :])
                    # reciprocal for this unit (both heads share a
                    # 64-partition band and column slot)
                    band = 32 * (hA % 4)
                    _, sc = sum_slot(hA, sqb)
                    nc.vector.reciprocal(
                        recip_t[band:band + 64, sc],
                        sums_t[band:band + 64, sc])
                    # normalize y^T for this unit in place
                    for h2, h in ((0, hA), (1, hB)):
                        rows = slice(64 * h2, 64 * h2 + 64)
                        sp, _ = sum_slot(h, sqb)
                        # HW partition_broadcast reads partition 0 of the
                        # tensor regardless of the AP base -> stage the
                        # recip row to partition 0 (cast to bf16) first.
                        rtmp = stage_pool.tile([128, 512], FP, tag="rtmp",
                                               name="rtmp")
                        nc.sync.dma_start(rtmp[0:1, :],
                                          recip_t[sp:sp + 1, sc])
                        rtb = stage_pool.tile([128, 512], BF, tag="rtb",
                                              name="rtb")
                        nc.vector.tensor_copy(rtb[0:1, :], rtmp[0:1, :])
                        rb = stage_pool.tile([128, 512], BF, tag="rb",
                                             name="rb")
                        nc.gpsimd.partition_broadcast(rb[:], rtb[0:1, :])
                        nc.vector.tensor_mul(
                            yT[hp][rows, sq], yT[hp][rows, sq],
                            rb[rows, :])
                    # ship + AllGather this (head-pair, quarter) now
                    nc.sync.dma_start(y_bnc[hp][sqb][:, :], yT[hp][:, sq])
                    nc.gpsimd.collective_compute(
                        "AllGather", mybir.AluOpType.bypass,
                        replica_groups=[[0, 1], [2, 3], [4, 5], [6, 7]],
                        ins=[y_bnc[hp][sqb][:, :]],
                        outs=[y_gath[hp][sqb][:, :]])

            # ---- tail: last seq-quarter of the output projection ----
            q = SQB - 1
            ygq = start_quarter(q)
            for tl in range(4):
                emit_outproj_ti(ygq, q, tl)


def build_program(S=2048):
    nc = bacc.Bacc(
        "TRN2",
        target_bir_lowering=False,
        debug=False,
        enable_asserts=True,
        num_devices=NCORES,
    )
    io = {
        "xt": nc.declare_dram_parameter("xt", [D, S], BF, isOutput=False),
        "wq": nc.declare_dram_parameter("wq", [D, DQ], BF, isOutput=False),
        "bqk": nc.declare_dram_parameter("bqk", [128, 8], FP, isOutput=False),
        "wk": nc.declare_dram_parameter("wk", [D, DQ], BF, isOutput=False),
        "wv": nc.declare_dram_parameter("wv", [D, DQ], BF, isOutput=False),
        "bv": nc.declare_dram_parameter("bv", [DQ], FP, isOutput=False),
        "wo": nc.declare_dram_parameter("wo", [D, DQ], BF, isOutput=False),
        "bo": nc.declare_dram_parameter("bo", [DQ], FP, isOutput=False),
        "out": nc.declare_dram_parameter("out", [S, DQ], FP, isOutput=True),
    }
    io = {k: (v[:] if not isinstance(v, bass.AP) else v) for k, v in io.items()}
    with tile.TileContext(nc) as tc:
        emit_mha(nc, tc, io, S)
    nc.finalize()
    return nc


def shard_inputs(x, Wq, bq, Wk, bk, Wv, bv, Wo, bo):
    """Full inputs -> per-core in_maps. Matmul operands cast to bf16; x is
    transposed on the host (input prep for the d-major device layout)."""
    BFNP = ml_dtypes.bfloat16
    f32 = lambda a: np.ascontiguousarray(np.asarray(a), dtype=np.float32)
    bf = lambda a: np.ascontiguousarray(np.asarray(a, dtype=np.float32)
                                        .astype(BFNP))
    x = np.asarray(x, dtype=np.float32).astype(BFNP)
    xts = [np.ascontiguousarray(x[b].T) for b in range(4)]
    Wq, Wk, Wv, Wo = bf(Wq), bf(Wk), bf(Wv), bf(Wo)
    bq, bk, bv, bo = f32(bq), f32(bk), f32(bv), f32(bo)
    in_maps = []
    for c in range(NCORES):
        b, g = divmod(c, 2)
        sl = slice(g * DQ, (g + 1) * DQ)
        bqk = np.empty((128, 8), np.float32)
        for m in range(4):
            bqk[:, m] = bq[sl][m * 128:(m + 1) * 128]
            bqk[:, 4 + m] = bk[sl][m * 128:(m + 1) * 128]
        in_maps.append({
            "xt": xts[b],
            "wq": np.ascontiguousarray(Wq[:, sl]), "bqk": bqk,
            "wk": np.ascontiguousarray(Wk[:, sl]),
            "wv": np.ascontiguousarray(Wv[:, sl]), "bv": bv[sl].copy(),
            "wo": np.ascontiguousarray(Wo[:, sl]), "bo": bo[sl].copy(),
        })
    return in_maps


_CACHE = {}


def _get_program(S=2048):
    if S not in _CACHE:
        _CACHE[S] = build_program(S)
    return _CACHE[S]


def kernel(x, Wq, bq, Wk, bk, Wv, bv, Wo, bo):
    nc = _get_program(2048)
    in_maps = shard_inputs(x, Wq, bq, Wk, bk, Wv, bv, Wo, bo)
    res = run_bass_kernel_spmd(nc, in_maps, list(range(NCORES))).results
    S = 2048
    out = np.empty((4, S, D), dtype=np.float32)
    for c in range(NCORES):
        b, g = divmod(c, 2)
        out[b, :, g * DQ:(g + 1) * DQ] = res[c]["out"]
    return out
